# revision 16
# baseline (speedup 1.0000x reference)
"""Trainium2 Bass kernel for CwRNN (nn_CwRNN_84971632984686).

Data-parallel over batch (64/8 = 8 rows per core). Per core:
- Host pre-transposes x to x^T [2ic, 128i, T, 8b] fp16 (plus a T/4-rate
  mid tensor) and pre-transposes all weights, so the device does zero
  transposes/casts; y is produced transposed fp16 and the host restores
  [b, T, I] fp32.
- Module-decoupled clockwork solve (block-triangular W_hh): module m
  depends only on modules >= m. Self-recurrence solved per 128-entry
  window by parallel-in-time Jacobi (K tanh passes, delta-accumulated
  in a persistent PSUM window).
- Windows use a ZERO boundary (fully independent, schedulable in
  lockstep pairs to keep ACT/PE saturated); a tiny FIX-entry fixup pass
  with the exact boundary restores the first entries afterwards.
  Boundary influence decays ~0.45^k so entries >= FIX are unaffected.
- Output y^T built per 64-step chunk in PSUM: G_m = fc_w,m^T @ v_m plus
  an upsample-add of the parent coarse level (c-hierarchy kept in SBUF
  fp16; the up2-add is fused into the PSUM->SBUF copy on DVE/Pool).
- Columns are entry-major (col = k*8 + b) so all broadcasts/upsamples
  are uniform access patterns.
"""
import os
import sys
import numpy as np

for _p in ("/root/.axon_site/_ro/trn_rl_repo", "/opt/trn_rl_repo"):
    if os.path.isdir(_p) and _p not in sys.path:
        sys.path.insert(0, _p)

import concourse.bass as bass  # noqa: E402
import concourse.mybir as mybir  # noqa: E402
from concourse import bacc  # noqa: E402
from concourse.tile import TileContext  # noqa: E402
from concourse.masks import make_identity  # noqa: E402
from concourse.bass_utils import run_bass_kernel_spmd  # noqa: E402

F32 = mybir.dt.float32
F16 = mybir.dt.float16
TANH = mybir.ActivationFunctionType.Tanh
ADD = mybir.AluOpType.add
SUB = mybir.AluOpType.subtract

CORES = 8
B, T, I, H, M = 64, 2048, 256, 1024, 8
MS = H // M          # 128, module size
BC = B // CORES      # 8 batch rows per core
P = 128
LE = 128             # max entries per solve window
K = 4                # Jacobi sweeps (tanh passes)
FIX = 8              # fixup entries per window boundary
KF = 3               # fixup sweeps
SPAN = 128           # steps per x/y span tile
GRP = 2              # windows solved in lockstep
TM4 = T // 4

_WIDX = {}
for _m in range(M):
    for _j in range(_m, M):
        _WIDX[(_j, _m)] = len(_WIDX)
NBLK = len(_WIDX)


def _windows():
    ws = []
    for m in range(M):
        Tm = T >> m
        L = min(LE, Tm)
        for w in range(Tm // L):
            ws.append((m, w, w * L, L))
    ws.sort(key=lambda s: (s[2] * (1 << s[0]), -s[0]))
    return ws


def build_nc():
    nc = bacc.Bacc("TRN2", target_bir_lowering=False, debug=False)
    dr = {}
    dr["x"] = nc.dram_tensor("x", [2, P, T, BC], F16, kind="ExternalInput")
    dr["xmid"] = nc.dram_tensor("xmid", [2, P, TM4, BC], F16,
                                kind="ExternalInput")
    dr["wihT"] = nc.dram_tensor("wihT", [P, 2, M, P], F16,
                                kind="ExternalInput")
    dr["whhT"] = nc.dram_tensor("whhT", [P, NBLK, P], F16,
                                kind="ExternalInput")
    dr["fcwT"] = nc.dram_tensor("fcwT", [P, M, 2, P], F16,
                                kind="ExternalInput")
    dr["biasb"] = nc.dram_tensor("biasb", [P, M], F32, kind="ExternalInput")
    dr["fcbb"] = nc.dram_tensor("fcbb", [P, 2], F32, kind="ExternalInput")
    dr["y"] = nc.dram_tensor("y", [2, P, T, BC], F16, kind="ExternalOutput")
    with TileContext(nc) as tc:
        _emit(tc, nc, dr)
    nc.compile()
    return nc


def _emit(tc, nc, dr):
    import contextlib
    ctx = contextlib.ExitStack()
    with ctx:
        cst = ctx.enter_context(tc.tile_pool(name="cst", bufs=1))
        xsp_pool = ctx.enter_context(tc.tile_pool(name="xsp", bufs=8))
        vper_pool = ctx.enter_context(tc.tile_pool(name="vper", bufs=1))
        v0f_pool = ctx.enter_context(tc.tile_pool(name="v0f", bufs=6))
        sw_pool = ctx.enter_context(tc.tile_pool(name="sw", bufs=3))
        fx_pool = ctx.enter_context(tc.tile_pool(name="fx", bufs=3))
        cch_pool = ctx.enter_context(tc.tile_pool(name="cch", bufs=4))
        yst_pool = ctx.enter_context(tc.tile_pool(name="yst", bufs=2))
        pw = ctx.enter_context(tc.tile_pool(name="pw", bufs=4, space="PSUM"))

        # ---------------- constants (no transposes: host pre-transposed) ---
        ident = cst.tile([P, P], F32)
        make_identity(nc, ident)
        ident16 = cst.tile([P, P], F16)
        nc.vector.tensor_copy(ident16[:], ident[:])

        wihT = cst.tile([P, 2, M, P], F16)
        nc.sync.dma_start(wihT[:], dr["wihT"][:, :, :, :])
        whhT = cst.tile([P, NBLK, P], F16)
        nc.sync.dma_start(whhT[:], dr["whhT"][:, :, :])
        fcwT = cst.tile([P, M, 2, P], F16)
        nc.sync.dma_start(fcwT[:], dr["fcwT"][:, :, :, :])
        bias_sb = cst.tile([P, M], F32)
        nc.sync.dma_start(bias_sb[:], dr["biasb"][:, :])
        fcb_sb = cst.tile([P, 2], F32)
        nc.sync.dma_start(fcb_sb[:], dr["fcbb"][:, :])
        xmid = cst.tile([P, 2, TM4 * BC], F16)
        nc.sync.dma_start(
            xmid[:], dr["xmid"][:, :, :, :].rearrange("a p t b -> p a (t b)"))

        # persistent per-module finals, slot s = entry s-1 (slot 0 = zero)
        vper = {}
        for m in range(1, M):
            Tm = T >> m
            vper[m] = vper_pool.tile([P, (Tm + 1) * BC], F16, tag=f"vp{m}",
                                     name=f"vp{m}")
            nc.vector.memset(vper[m][:, 0:BC], 0.0)

        # coarse output levels for single-window modules (computed once)
        csing = {}
        for m in range(4, M):
            Tm = T >> m
            csing[m] = cst.tile([P, 2, Tm * BC], F32, name=f"c{m}")

        xtiles = {}

        def load_span(s):
            if s in xtiles:
                return
            t0 = xsp_pool.tile([P, 2, SPAN * BC], F16, tag="xsp", name="xt")
            nc.sync.dma_start(
                t0[:],
                dr["x"][:, :, s * SPAN:(s + 1) * SPAN, :].rearrange(
                    "a p t b -> p a (t b)"))
            xtiles[s] = t0

        v0fin = {}      # w -> m=0 final tile [P, L*BC] (entry k at col k*8)
        cchunk = {}     # (m, w) -> c_m chunk tile [P, 2, L*BC]
        copy_ctr = [0]

        def copy_engine():
            # Pool/GPSIMD cannot access PSUM on HW; DVE handles fused copies
            copy_ctr[0] += 1
            return nc.vector

        def u_rhs(m, w, ic, ka, kb):
            """x^T RHS for window entries [ka, kb) (module-local)."""
            k0 = w * (min(LE, T >> m))
            if m == 0:
                xs = xtiles[w][:, ic, :].rearrange("p (t b) -> p t b", b=BC)
                return xs[:, ka:kb, :]
            if m == 1:
                tile = xtiles[2 * w + (ka // 64)]
                xs = tile[:, ic, :].rearrange("p (t s b) -> p t s b",
                                              s=2, b=BC)
                return xs[:, 0:kb - ka, 0, :]
            stride = 1 << (m - 2)
            xm = xmid[:, ic, :].rearrange("p (k s b) -> p k s b",
                                          s=stride, b=BC)
            return xm[:, k0 + ka:k0 + kb, 0, :]

        def emit_C(m, w, k0, L, Pf, started):
            """P[:, k*8+b] += sum_{j>m} W_mj v_j[slot k0/r + ceil(k/r)]."""
            groups = [(0, min(64, L))] + ([(64, L)] if L > 64 else [])
            for j in range(m + 1, M):
                r = 1 << (j - m)
                sb = k0 // r
                lhsT = whhT[:, _WIDX[(j, m)], :]
                Vj = vper[j][:].rearrange("p (s b) -> p s b", b=BC)
                for (ka, kb) in groups:
                    gi = ka // 64

                    def mm(c0, c1, rhs):
                        st = gi not in started
                        if st:
                            started.add(gi)
                        nc.tensor.matmul(Pf[:, c0 * BC:c1 * BC], lhsT, rhs,
                                         start=st, stop=False,
                                         skip_group_check=True)

                    if ka == 0:
                        mm(0, 1, Vj[:, sb:sb + 1, :])
                    k = max(ka, 1)
                    # runs: slot ceil(k/r) covers ks ((s-1)r, s*r]
                    s_lo = (k + r - 1) // r
                    head_end = min(kb, (s_lo - 1) * r + r + 1)
                    if head_end - k < r:  # partial head
                        rhs = Vj[:, sb + s_lo:sb + s_lo + 1, :][
                            :, :, None, :].broadcast_to(
                                (P, 1, head_end - k, BC))
                        mm(k, head_end, rhs)
                        k = head_end
                        s_lo += 1
                    if k < kb:
                        nfull = (kb - k) // r
                        if nfull:
                            rhs = Vj[:, sb + s_lo:sb + s_lo + nfull, :][
                                :, :, None, :].broadcast_to(
                                    (P, nfull, r, BC))
                            mm(k, k + nfull * r, rhs)
                            k += nfull * r
                            s_lo += nfull
                        if k < kb:  # partial tail
                            rhs = Vj[:, sb + s_lo:sb + s_lo + 1, :][
                                :, :, None, :].broadcast_to(
                                    (P, 1, kb - k, BC))
                            mm(k, kb, rhs)

        def emit_U(m, w, k0, L):
            Pp = pw.tile([P, LE * BC], F32, tag="pw",
                         name=f"P{m}_{w}")[:, :L * BC]
            started = set()
            groups = [(0, min(64, L))] + ([(64, L)] if L > 64 else [])
            if m == 0:
                load_span(w)
            elif m == 1:
                load_span(2 * w)
                load_span(2 * w + 1)
            for ic in range(2):
                for (ka, kb) in groups:
                    gi = ka // 64
                    st = gi not in started
                    if st:
                        started.add(gi)
                    nc.tensor.matmul(Pp[:, ka * BC:kb * BC],
                                     wihT[:, ic, m, :], u_rhs(m, w, ic, ka, kb),
                                     start=st, stop=False,
                                     skip_group_check=True)
            return Pp, started

        def emit_Cfix(m, w, k0, L, Pp, started):
            emit_C(m, w, k0, L, Pp, started)
            fixS = None
            if w > 0:
                fixS = fx_pool.tile([P, FIX * BC], F16, tag="fS", name="fS")
                nc.vector.tensor_copy(fixS[:], Pp[:, :FIX * BC])
            return fixS

        def emit_fixup(m, w, k0, L, Pp):
            """Redo entries [0..FIX) with exact boundary from window w-1.
            Reuses the window's own (dead) PSUM columns for the fixup."""
            if m == 0:
                bnd = v0fin[w - 1][:, (L - 1) * BC:L * BC]
                main = v0fin[w]
                main_head = main[:, :(FIX - 1) * BC]
                fin = main[:, :FIX * BC]
            else:
                Vm = vper[m]
                bnd = Vm[:, k0 * BC:(k0 + 1) * BC]
                main_head = Vm[:, (k0 + 1) * BC:(k0 + FIX) * BC]
                fin = Vm[:, (k0 + 1) * BC:(k0 + 1 + FIX) * BC]
            fixS = fixS_of.pop((m, w))
            lhsT = whhT[:, _WIDX[(m, m)], :]
            bias = bias_sb[:, m:m + 1]
            PF = Pp[:, :FIX * BC]
            nc.tensor.matmul(PF, ident16[:], fixS[:], start=True,
                             stop=False, skip_group_check=True)
            nc.tensor.matmul(PF[:, 0:BC], lhsT, bnd, start=False, stop=False,
                             skip_group_check=True)
            nc.tensor.matmul(PF[:, BC:FIX * BC], lhsT, main_head,
                             start=False, stop=False, skip_group_check=True)
            fA = fx_pool.tile([P, FIX * BC], F16, tag="fA", name="fA")
            fB = fx_pool.tile([P, FIX * BC], F16, tag="fB", name="fB")
            fD = fx_pool.tile([P, (FIX - 1) * BC], F16, tag="fD", name="fD")
            nc.scalar.activation(fA[:], PF, TANH, bias=bias, scale=1.0)
            prev_src, cur_buf = main_head, fA
            for it in range(2, KF + 1):
                last = it == KF
                nc.vector.tensor_tensor(fD[:], cur_buf[:, :(FIX - 1) * BC],
                                        prev_src, SUB)
                nc.tensor.matmul(PF[:, BC:FIX * BC], lhsT, fD[:],
                                 start=False, stop=last,
                                 skip_group_check=True)
                out = fin if last else (fB if cur_buf is fA else fA)
                nc.scalar.activation(out, PF, TANH, bias=bias, scale=1.0)
                prev_src = cur_buf[:, :(FIX - 1) * BC]
                cur_buf = out

        def emit_output_items(m, w, k0, L, Pp):
            """Returns closures: G_m chunks + fused up2-add into c_m / y
            (reuses the window's PSUM)."""
            ngr = max(1, L // 64)
            if m == 0:
                dst = yst_pool.tile([P, 2, SPAN * BC], F16, tag="yst",
                                    name=f"yst{w}")
                par = cchunk[(1, w // 2)]
                pk0 = (w // 2) * LE
            elif m >= 4:
                dst = csing[m]
                par = csing[m + 1] if m < M - 1 else None
                pk0 = 0
            else:
                dst = cch_pool.tile([P, 2, L * BC], F16, tag=f"c{m}",
                                    name=f"c{m}_{w}")
                cchunk[(m, w)] = dst
                par = csing[4] if m == 3 else cchunk[(m + 1, w // 2)]
                pk0 = 0 if m == 3 else (w // 2) * LE
            items = []
            for ic in range(2):
                for g in range(ngr):
                    def chunk(ic=ic, g=g):
                        ka, kb = 64 * g, min(64 * (g + 1), L)
                        ncols = (kb - ka) * BC
                        off = (512 * ((ic * ngr + g) % 2)
                               if L * BC >= 1024 else 0)
                        g_ps = Pp[:, off:off + ncols]
                        if m == 0:
                            rhs = v0fin[w][:, ka * BC:kb * BC]
                        else:
                            rhs = vper[m][:, (k0 + 1 + ka) * BC:
                                          (k0 + 1 + kb) * BC]
                        nc.tensor.matmul(g_ps, fcwT[:, m, ic, :], rhs,
                                         start=True, stop=True,
                                         skip_group_check=True)
                        out = dst[:, ic, ka * BC:kb * BC]
                        if m == M - 1:
                            nc.vector.tensor_scalar_add(out, g_ps,
                                                        fcb_sb[:, ic:ic + 1])
                        else:
                            e0 = (k0 + ka) // 2 - pk0
                            ne = (kb - ka) // 2
                            pv = par[:, ic, :].rearrange("p (e b) -> p e b",
                                                         b=BC)
                            rhs2 = pv[:, e0:e0 + ne, :][:, :, None, :] \
                                .broadcast_to((P, ne, 2, BC))
                            copy_engine().tensor_tensor(out, g_ps, rhs2, ADD)
                    items.append(chunk)
            if m == 0:
                def ydma():
                    nc.gpsimd.dma_start(
                        dr["y"][:, :, w * SPAN:(w + 1) * SPAN, :].rearrange(
                            "a p t b -> p a (t b)"), dst[:])
                items.append(ydma)
            return items

        # -------- main loop: 3-stage pipeline with gap-filling interleave ----
        # iteration i: [deferred C(i-1)] | sweeps(i-1) with fillers:
        #   stage1 -> output chunks(i-2), stage2 -> U(i),
        #   stage3 -> C(i) for windows whose producers are already fixed
        # | tail: fixups(i-1).
        fixS_of = {}
        psums_of = {}
        started_of = {}
        fix_done = set()
        c_done = set()
        wins = _windows()
        emitted = set()
        order = []
        by_mw = {(m, w): (m, w, k0, L) for (m, w, k0, L) in wins}
        for (m, w, k0, L) in wins:
            if (m, w) in emitted:
                continue
            grp = [(m, w, k0, L)]
            emitted.add((m, w))
            nxt = (m, w + 1)
            if len(grp) < GRP and nxt in by_mw and nxt not in emitted:
                grp.append(by_mw[nxt])
                emitted.add(nxt)
            order.append(grp)
        NG = len(order)

        def producers_fixed(m, k0, L):
            for j in range(m + 1, M):
                r = 1 << (j - m)
                Lj = min(LE, T >> j)
                e_lo = max(0, k0 // r - 1)
                e_hi = (k0 + L) // r - 1
                for wj in range(e_lo // Lj, e_hi // Lj + 1):
                    if (j, wj) not in fix_done:
                        return False
            return True

        def do_Cfix(m, w, k0, L):
            fixS = emit_Cfix(m, w, k0, L, psums_of[(m, w)],
                             started_of.pop((m, w)))
            if fixS is not None:
                fixS_of[(m, w)] = fixS
            c_done.add((m, w))

        def do_fixups(g):
            for (m, w, k0, L) in order[g]:
                if w > 0:
                    emit_fixup(m, w, k0, L, psums_of[(m, w)])
                fix_done.add((m, w))

        def emit_all(items):
            for f in items:
                f()

        for i in range(NG + 2):
            out_items = []
            if i >= 2:
                its = []
                for (m, w, k0, L) in order[i - 2]:
                    its.append(emit_output_items(m, w, k0, L,
                                                 psums_of.pop((m, w))))
                while any(its):
                    for lst in its:
                        if lst:
                            out_items.append(lst.pop(0))

            if 1 <= i <= NG:
                grp = order[i - 1]
                for (m, w, k0, L) in grp:
                    if (m, w) not in c_done:
                        do_Cfix(m, w, k0, L)
                gens = [
                    _sweep_gen(nc, m, w, k0, L, psums_of[(m, w)], bias_sb,
                               whhT, sw_pool, v0f_pool, vper, v0fin)
                    for (m, w, k0, L) in grp
                ]
                for it in range(1, K + 1):
                    for g in gens:
                        next(g, None)
                    if it == 1:
                        half = (len(out_items) + 1) // 2
                        for f in out_items[:half]:
                            f()
                        out_items = out_items[half:]
                    elif it == 2:
                        emit_all(out_items)
                        out_items = []
                    elif it == 3 and i < NG:
                        for (m, w, k0, L) in order[i]:
                            psums_of[(m, w)], started_of[(m, w)] = \
                                emit_U(m, w, k0, L)
                    elif it == 4 and i < NG:
                        for (m, w, k0, L) in order[i]:
                            if producers_fixed(m, k0, L):
                                do_Cfix(m, w, k0, L)
                do_fixups(i - 1)
            else:
                emit_all(out_items)
                if i < NG:
                    for (m, w, k0, L) in order[i]:
                        psums_of[(m, w)], started_of[(m, w)] = \
                            emit_U(m, w, k0, L)

def _sweep_gen(nc, m, w, k0, L, Pp, bias_sb, whhT, sw_pool, v0f_pool,
               vper, v0fin):
    """Generator emitting one sweep stage per next() for lockstep pairing."""
    bias = bias_sb[:, m:m + 1]
    if m == 0:
        vfin = v0f_pool.tile([P, LE * BC], F16, tag="v0f",
                             name=f"v0f{w}")[:, :L * BC]
        v0fin[w] = vfin
        fin_ap = vfin
    else:
        fin_ap = vper[m][:, (k0 + 1) * BC:(k0 + 1 + L) * BC]
    sA = sw_pool.tile([P, LE * BC], F16, tag="swA", name=f"sA{m}_{w}")[:, :L * BC]
    sB = sw_pool.tile([P, LE * BC], F16, tag="swB", name=f"sB{m}_{w}")[:, :L * BC]
    sD = sw_pool.tile([P, LE * BC], F16, tag="swD", name=f"sD{m}_{w}")[:, :L * BC]
    lhsT = whhT[:, _WIDX[(m, m)], :]
    mm_groups = [(1, min(64, L))] + ([(64, L)] if L > 64 else [])

    def sweep_mm(rhs_buf, last):
        for (ka, kb) in mm_groups:
            nc.tensor.matmul(Pp[:, ka * BC:kb * BC], lhsT,
                             rhs_buf[:, (ka - 1) * BC:(kb - 1) * BC],
                             start=False, stop=last, skip_group_check=True)

    prev, cur = None, None  # v^{i-2}, v^{i-1} buffers
    for it in range(1, K + 1):
        last = it == K
        if it == 2:
            sweep_mm(cur, last)
        elif it > 2:
            nc.vector.tensor_tensor(sD[:, :(L - 1) * BC],
                                    cur[:, :(L - 1) * BC],
                                    prev[:, :(L - 1) * BC], SUB)
            sweep_mm(sD, last)
        out = fin_ap if last else (sA if it % 2 else sB)
        nc.scalar.activation(out, Pp[:, :L * BC], TANH, bias=bias, scale=1.0)
        prev, cur = cur, out
        yield


_NC_CACHE = None


def _prep_weights(inputs):
    wih = np.asarray(inputs["weight_ih"], dtype=np.float32)
    whh = np.asarray(inputs["weight_hh"], dtype=np.float32)
    fcw = np.asarray(inputs["fc_w"], dtype=np.float32)
    wihT = np.ascontiguousarray(
        wih.reshape(M, MS, 2, P).transpose(3, 2, 0, 1).astype(np.float16))
    whhT = np.empty((P, NBLK, P), dtype=np.float16)
    for (j, m), idx in _WIDX.items():
        whhT[:, idx, :] = whh[m * MS:(m + 1) * MS, j * MS:(j + 1) * MS].T
    fcwT = np.ascontiguousarray(
        fcw.reshape(2, P, M, MS).transpose(3, 2, 0, 1).astype(np.float16))
    biasb = np.ascontiguousarray(
        (np.asarray(inputs["bias_ih"], dtype=np.float32)
         + np.asarray(inputs["bias_hh"], dtype=np.float32))
        .reshape(M, P).T)
    fcbb = np.ascontiguousarray(
        np.asarray(inputs["fc_b"], dtype=np.float32).reshape(2, P).T)
    return dict(wihT=wihT, whhT=whhT, fcwT=fcwT, biasb=biasb, fcbb=fcbb)


def _prep_x(x_core):
    """[BC, T, I] fp32 -> x^T [2, P, T, BC] fp16 (+ mid-rate tensor)."""
    xt = np.ascontiguousarray(
        x_core.transpose(2, 1, 0).astype(np.float16).reshape(2, P, T, BC))
    xmid = np.ascontiguousarray(xt[:, :, ::4, :])
    return xt, xmid


def kernel(**inputs):
    global _NC_CACHE
    x = np.asarray(inputs["x"], dtype=np.float32)
    assert int(np.asarray(inputs["n_modules"])) == M
    weights = _prep_weights(inputs)
    if _NC_CACHE is None:
        _NC_CACHE = build_nc()
    nc = _NC_CACHE
    in_maps = []
    for c in range(CORES):
        xt, xmid = _prep_x(x[c * BC:(c + 1) * BC])
        in_maps.append(dict(x=xt, xmid=xmid, **weights))
    res = run_bass_kernel_spmd(nc, in_maps, list(range(CORES)))
    out = np.empty((B, T, I), dtype=np.float32)
    for c in range(CORES):
        yt = res.results[c]["y"]  # [2, P, T, BC] fp16
        out[c * BC:(c + 1) * BC] = \
            yt.reshape(I, T, BC).transpose(2, 1, 0).astype(np.float32)
    return out


if __name__ == "__main__":
    build_nc()
    print("built OK")


# revision 23
# speedup vs baseline: 1.0574x; 1.0574x over previous
"""Trainium2 Bass kernel for CwRNN (nn_CwRNN_84971632984686).

Data-parallel over batch (64/8 = 8 rows per core). Per core:
- Host pre-transposes x to x^T [2ic, 128i, T, 8b] fp16 (plus a T/4-rate
  mid tensor) and pre-transposes all weights, so the device does zero
  transposes/casts; y is produced transposed fp16 and the host restores
  [b, T, I] fp32.
- Module-decoupled clockwork solve (block-triangular W_hh): module m
  depends only on modules >= m. Self-recurrence solved per 128-entry
  window by parallel-in-time Jacobi (K tanh passes, delta-accumulated
  in a persistent PSUM window).
- Windows use a ZERO boundary (fully independent, schedulable in
  lockstep pairs to keep ACT/PE saturated); a tiny FIX-entry fixup pass
  with the exact boundary restores the first entries afterwards.
  Boundary influence decays ~0.45^k so entries >= FIX are unaffected.
- Output y^T built per 64-step chunk in PSUM: G_m = fc_w,m^T @ v_m plus
  an upsample-add of the parent coarse level (c-hierarchy kept in SBUF
  fp16; the up2-add is fused into the PSUM->SBUF copy on DVE/Pool).
- Columns are entry-major (col = k*8 + b) so all broadcasts/upsamples
  are uniform access patterns.
"""
import os
import sys
import numpy as np

for _p in ("/root/.axon_site/_ro/trn_rl_repo", "/opt/trn_rl_repo"):
    if os.path.isdir(_p) and _p not in sys.path:
        sys.path.insert(0, _p)

import concourse.bass as bass  # noqa: E402
import concourse.mybir as mybir  # noqa: E402
from concourse import bacc  # noqa: E402
from concourse.tile import TileContext  # noqa: E402
from concourse.masks import make_identity  # noqa: E402
from concourse.bass_utils import run_bass_kernel_spmd  # noqa: E402

F32 = mybir.dt.float32
F16 = mybir.dt.float16
TANH = mybir.ActivationFunctionType.Tanh
COPYF = mybir.ActivationFunctionType.Copy
ADD = mybir.AluOpType.add
SUB = mybir.AluOpType.subtract

CORES = 8
B, T, I, H, M = 64, 2048, 256, 1024, 8
MS = H // M          # 128, module size
BC = B // CORES      # 8 batch rows per core
P = 128
LE = 128             # max entries per solve window
K = 4                # Jacobi sweeps (tanh passes)
FIX = 8              # fixup entries per window boundary
KF = 3               # fixup sweeps
SPAN = 128           # steps per x/y span tile
GRP = 2              # windows solved in lockstep
TM4 = T // 4

_WIDX = {}
for _m in range(M):
    for _j in range(_m, M):
        _WIDX[(_j, _m)] = len(_WIDX)
NBLK = len(_WIDX)


def _run_segments(ka, kb, r):
    """Segments covering ks [max(ka,1), kb): list of (k_start, slot_start,
    n, rep): out cols [k_start, k_start+n*rep) read slots [slot_start,
    slot_start+n) broadcast rep-wide. ceil(k/r) indexing."""
    segs = []
    k = max(ka, 1)
    while k < kb:
        s = (k + r - 1) // r
        run_end = min(kb, s * r + 1)
        ln = run_end - k
        if ln == r:
            nfull = (kb - k) // r
            segs.append((k, s, nfull, r))
            k += nfull * r
        else:
            segs.append((k, s, 1, ln))
            k = run_end
    return segs


def _windows():
    ws = []
    for m in range(M):
        Tm = T >> m
        L = min(LE, Tm)
        for w in range(Tm // L):
            ws.append((m, w, w * L, L))
    ws.sort(key=lambda s: (s[2] * (1 << s[0]), -s[0]))
    return ws


def build_nc():
    nc = bacc.Bacc("TRN2", target_bir_lowering=False, debug=False)
    dr = {}
    dr["x"] = nc.dram_tensor("x", [2, P, T, BC], F16, kind="ExternalInput")
    dr["xmid"] = nc.dram_tensor("xmid", [2, P, TM4, BC], F16,
                                kind="ExternalInput")
    dr["wihT"] = nc.dram_tensor("wihT", [P, 2, M, P], F16,
                                kind="ExternalInput")
    dr["whhT"] = nc.dram_tensor("whhT", [P, NBLK, P], F16,
                                kind="ExternalInput")
    dr["fcwT"] = nc.dram_tensor("fcwT", [P, M, 2, P], F16,
                                kind="ExternalInput")
    dr["biasb"] = nc.dram_tensor("biasb", [P, M], F32, kind="ExternalInput")
    dr["fcbb"] = nc.dram_tensor("fcbb", [P, 2], F32, kind="ExternalInput")
    dr["y"] = nc.dram_tensor("y", [2, P, T, BC], F16, kind="ExternalOutput")
    with TileContext(nc) as tc:
        _emit(tc, nc, dr)
    nc.compile()
    return nc


def _emit(tc, nc, dr):
    import contextlib
    ctx = contextlib.ExitStack()
    with ctx:
        cst = ctx.enter_context(tc.tile_pool(name="cst", bufs=1))
        xsp_pool = ctx.enter_context(tc.tile_pool(name="xsp", bufs=8))
        vper_pool = ctx.enter_context(tc.tile_pool(name="vper", bufs=1))
        v0f_pool = ctx.enter_context(tc.tile_pool(name="v0f", bufs=6))
        sw_pool = ctx.enter_context(tc.tile_pool(name="sw", bufs=3))
        fx_pool = ctx.enter_context(tc.tile_pool(name="fx", bufs=3))
        cch_pool = ctx.enter_context(tc.tile_pool(name="cch", bufs=4))
        yst_pool = ctx.enter_context(tc.tile_pool(name="yst", bufs=2))
        pw = ctx.enter_context(tc.tile_pool(name="pw", bufs=4, space="PSUM"))

        # ---------------- constants (no transposes: host pre-transposed) ---
        ident = cst.tile([P, P], F32)
        make_identity(nc, ident)
        ident16 = cst.tile([P, P], F16)
        nc.vector.tensor_copy(ident16[:], ident[:])

        wihT = cst.tile([P, 2, M, P], F16)
        nc.sync.dma_start(wihT[:], dr["wihT"][:, :, :, :])
        whhT = cst.tile([P, NBLK, P], F16)
        nc.sync.dma_start(whhT[:], dr["whhT"][:, :, :])
        fcwT = cst.tile([P, M, 2, P], F16)
        nc.sync.dma_start(fcwT[:], dr["fcwT"][:, :, :, :])
        bias_sb = cst.tile([P, M], F32)
        nc.sync.dma_start(bias_sb[:], dr["biasb"][:, :])
        fcb_sb = cst.tile([P, 2], F32)
        nc.sync.dma_start(fcb_sb[:], dr["fcbb"][:, :])
        xmid = cst.tile([P, 2, TM4 * BC], F16)
        xmv = dr["xmid"][:, :, :, :].rearrange("a p t b -> p a (t b)")
        nc.sync.dma_start(xmid[:, 0, :], xmv[:, 0, :])
        nc.scalar.dma_start(xmid[:, 1, :], xmv[:, 1, :])

        # persistent per-module finals, slot s = entry s-1 (slot 0 = zero)
        vper = {}
        for m in range(1, M):
            Tm = T >> m
            vper[m] = vper_pool.tile([P, (Tm + 1) * BC], F16, tag=f"vp{m}",
                                     name=f"vp{m}")
            nc.vector.memset(vper[m][:, 0:BC], 0.0)

        # coarse output levels for single-window modules (computed once)
        csing = {}
        for m in range(4, M):
            Tm = T >> m
            csing[m] = cst.tile([P, 2, Tm * BC], F32, name=f"c{m}")

        xtiles = {}

        def load_span(s):
            if s in xtiles:
                return
            t0 = xsp_pool.tile([P, 2, SPAN * BC], F16, tag="xsp", name="xt")
            nc.sync.dma_start(
                t0[:],
                dr["x"][:, :, s * SPAN:(s + 1) * SPAN, :].rearrange(
                    "a p t b -> p a (t b)"))
            xtiles[s] = t0

        v0fin = {}      # w -> m=0 final tile [P, L*BC] (entry k at col k*8)
        cchunk = {}     # (m, w) -> c_m chunk tile [P, 2, L*BC]
        copy_ctr = [0]

        def copy_engine():
            # Pool/GPSIMD cannot access PSUM on HW; DVE handles fused copies
            copy_ctr[0] += 1
            return nc.vector

        def u_rhs(m, w, ic, ka, kb):
            """x^T RHS for window entries [ka, kb) (module-local)."""
            k0 = w * (min(LE, T >> m))
            if m == 0:
                xs = xtiles[w][:, ic, :].rearrange("p (t b) -> p t b", b=BC)
                return xs[:, ka:kb, :]
            if m == 1:
                tile = xtiles[2 * w + (ka // 64)]
                xs = tile[:, ic, :].rearrange("p (t s b) -> p t s b",
                                              s=2, b=BC)
                return xs[:, 0:kb - ka, 0, :]
            stride = 1 << (m - 2)
            xm = xmid[:, ic, :].rearrange("p (k s b) -> p k s b",
                                          s=stride, b=BC)
            return xm[:, k0 + ka:k0 + kb, 0, :]

        def emit_C(m, w, k0, L, Pf, started):
            """P[:, k*8+b] += sum_{j>m} W_mj v_j[slot k0/r + ceil(k/r)]."""
            groups = [(0, min(64, L))] + ([(64, L)] if L > 64 else [])
            for j in range(m + 1, M):
                r = 1 << (j - m)
                sb = k0 // r
                lhsT = whhT[:, _WIDX[(j, m)], :]
                Vj = vper[j][:].rearrange("p (s b) -> p s b", b=BC)
                for (ka, kb) in groups:
                    gi = ka // 64

                    def mm(c0, c1, rhs):
                        st = gi not in started
                        if st:
                            started.add(gi)
                        nc.tensor.matmul(Pf[:, c0 * BC:c1 * BC], lhsT, rhs,
                                         start=st, stop=False,
                                         skip_group_check=True)

                    if ka == 0:
                        mm(0, 1, Vj[:, sb:sb + 1, :])
                    for (k, s, n, rep) in _run_segments(ka, kb, r):
                        rhs = Vj[:, sb + s:sb + s + n, :][
                            :, :, None, :].broadcast_to((P, n, rep, BC))
                        mm(k, k + n * rep, rhs)

        ch_of = {}

        def emit_Chalf_pre(m, w, k0, Pp):
            """Coarse C at half rate into Pp[:, 512:1024); copy to SBUF."""
            first = [True]
            for j in range(m + 1, M):
                rh = 1 << (j - m - 1)
                sb = k0 // (rh * 2)
                lhsT = whhT[:, _WIDX[(j, m)], :]
                Vj = vper[j][:].rearrange("p (s b) -> p s b", b=BC)
                for (i, s, n, rep) in _run_segments(1, 65, rh):
                    rhs = Vj[:, sb + s:sb + s + n, :][
                        :, :, None, :].broadcast_to((P, n, rep, BC))
                    nc.tensor.matmul(
                        Pp[:, 512 + (i - 1) * BC:512 + (i - 1 + n * rep) * BC],
                        lhsT, rhs, start=first[0], stop=False,
                        skip_group_check=True)
                    first[0] = False
            ch = fx_pool.tile([P, 512], F16, tag="ch", name="ch")
            nc.scalar.activation(ch[:], Pp[:, 512:1024], COPYF, bias=0.0,
                                 scale=1.0)
            ch_of[(m, w)] = ch

        def emit_Chalf_post(m, w, k0, Pp):
            """k=0 boundary singles + up2 of coarse C into the window."""
            ch = ch_of.pop((m, w))
            Chv = ch[:].rearrange("p (i b) -> p i b", b=BC)
            for j in range(m + 1, M):
                r = 1 << (j - m)
                sb = k0 // r
                lhsT = whhT[:, _WIDX[(j, m)], :]
                Vj = vper[j][:].rearrange("p (s b) -> p s b", b=BC)
                nc.tensor.matmul(Pp[:, 0:BC], lhsT, Vj[:, sb:sb + 1, :],
                                 start=False, stop=False,
                                 skip_group_check=True)
            for (ka, kb) in ((0, 64), (64, 128)):
                for (k, s, n, rep) in _run_segments(ka, kb, 2):
                    rhs = Chv[:, s - 1:s - 1 + n, :][
                        :, :, None, :].broadcast_to((P, n, rep, BC))
                    nc.tensor.matmul(Pp[:, k * BC:(k + n * rep) * BC],
                                     ident16[:], rhs, start=False,
                                     stop=False, skip_group_check=True)

        def emit_U(m, w, k0, L):
            Pp = pw.tile([P, LE * BC], F32, tag="pw",
                         name=f"P{m}_{w}")[:, :L * BC]
            started = set()
            groups = [(0, min(64, L))] + ([(64, L)] if L > 64 else [])
            if m == 0:
                load_span(w)
            elif m == 1:
                load_span(2 * w)
                load_span(2 * w + 1)
            half = m <= 3 and producers_fixed(m, k0, L)
            if half:
                emit_Chalf_pre(m, w, k0, Pp)
            for ic in range(2):
                for (ka, kb) in groups:
                    gi = ka // 64
                    st = gi not in started
                    if st:
                        started.add(gi)
                    nc.tensor.matmul(Pp[:, ka * BC:kb * BC],
                                     wihT[:, ic, m, :], u_rhs(m, w, ic, ka, kb),
                                     start=st, stop=False,
                                     skip_group_check=True)
            if half:
                emit_Chalf_post(m, w, k0, Pp)
                started = None
            return Pp, started

        def emit_Cfix(m, w, k0, L, Pp, started):
            if started is not None:
                emit_C(m, w, k0, L, Pp, started)
            fixS = None
            if w > 0:
                fixS = fx_pool.tile([P, FIX * BC], F16, tag="fS", name="fS")
                nc.vector.tensor_copy(fixS[:], Pp[:, :FIX * BC])
            return fixS

        def emit_fixup(m, w, k0, L, Pp):
            """Redo entries [0..FIX) with exact boundary from window w-1.
            Reuses the window's own (dead) PSUM columns for the fixup."""
            if m == 0:
                bnd = v0fin[w - 1][:, (L - 1) * BC:L * BC]
                main = v0fin[w]
                main_head = main[:, :(FIX - 1) * BC]
                fin = main[:, :FIX * BC]
            else:
                Vm = vper[m]
                bnd = Vm[:, k0 * BC:(k0 + 1) * BC]
                main_head = Vm[:, (k0 + 1) * BC:(k0 + FIX) * BC]
                fin = Vm[:, (k0 + 1) * BC:(k0 + 1 + FIX) * BC]
            fixS = fixS_of.pop((m, w))
            lhsT = whhT[:, _WIDX[(m, m)], :]
            bias = bias_sb[:, m:m + 1]
            PF = Pp[:, :FIX * BC]
            nc.tensor.matmul(PF, ident16[:], fixS[:], start=True,
                             stop=False, skip_group_check=True)
            nc.tensor.matmul(PF[:, 0:BC], lhsT, bnd, start=False, stop=False,
                             skip_group_check=True)
            nc.tensor.matmul(PF[:, BC:FIX * BC], lhsT, main_head,
                             start=False, stop=False, skip_group_check=True)
            fA = fx_pool.tile([P, FIX * BC], F16, tag="fA", name="fA")
            fB = fx_pool.tile([P, FIX * BC], F16, tag="fB", name="fB")
            fD = fx_pool.tile([P, (FIX - 1) * BC], F16, tag="fD", name="fD")
            nc.scalar.activation(fA[:], PF, TANH, bias=bias, scale=1.0)
            prev_src, cur_buf = main_head, fA
            for it in range(2, KF + 1):
                last = it == KF
                nc.vector.tensor_tensor(fD[:], cur_buf[:, :(FIX - 1) * BC],
                                        prev_src, SUB)
                nc.tensor.matmul(PF[:, BC:FIX * BC], lhsT, fD[:],
                                 start=False, stop=last,
                                 skip_group_check=True)
                out = fin if last else (fB if cur_buf is fA else fA)
                nc.scalar.activation(out, PF, TANH, bias=bias, scale=1.0)
                prev_src = cur_buf[:, :(FIX - 1) * BC]
                cur_buf = out

        def emit_output_items(m, w, k0, L, Pp):
            """Returns closures: G_m chunks + fused up2-add into c_m / y
            (reuses the window's PSUM)."""
            ngr = max(1, L // 64)
            if m == 0:
                dst = yst_pool.tile([P, 2, SPAN * BC], F16, tag="yst",
                                    name=f"yst{w}")
                par = cchunk[(1, w // 2)]
                pk0 = (w // 2) * LE
            elif m >= 4:
                dst = csing[m]
                par = csing[m + 1] if m < M - 1 else None
                pk0 = 0
            else:
                dst = cch_pool.tile([P, 2, L * BC], F16, tag=f"c{m}",
                                    name=f"c{m}_{w}")
                cchunk[(m, w)] = dst
                par = csing[4] if m == 3 else cchunk[(m + 1, w // 2)]
                pk0 = 0 if m == 3 else (w // 2) * LE
            items = []
            for ic in range(2):
                for g in range(ngr):
                    def chunk(ic=ic, g=g):
                        ka, kb = 64 * g, min(64 * (g + 1), L)
                        ncols = (kb - ka) * BC
                        off = (512 * ((ic * ngr + g) % 2)
                               if L * BC >= 1024 else 0)
                        g_ps = Pp[:, off:off + ncols]
                        if m == 0:
                            rhs = v0fin[w][:, ka * BC:kb * BC]
                        else:
                            rhs = vper[m][:, (k0 + 1 + ka) * BC:
                                          (k0 + 1 + kb) * BC]
                        nc.tensor.matmul(g_ps, fcwT[:, m, ic, :], rhs,
                                         start=True, stop=True,
                                         skip_group_check=True)
                        out = dst[:, ic, ka * BC:kb * BC]
                        if m == M - 1:
                            nc.vector.tensor_scalar_add(out, g_ps,
                                                        fcb_sb[:, ic:ic + 1])
                        else:
                            e0 = (k0 + ka) // 2 - pk0
                            ne = (kb - ka) // 2
                            pv = par[:, ic, :].rearrange("p (e b) -> p e b",
                                                         b=BC)
                            rhs2 = pv[:, e0:e0 + ne, :][:, :, None, :] \
                                .broadcast_to((P, ne, 2, BC))
                            copy_engine().tensor_tensor(out, g_ps, rhs2, ADD)
                    items.append(chunk)
            if m == 0:
                def ydma():
                    nc.gpsimd.dma_start(
                        dr["y"][:, :, w * SPAN:(w + 1) * SPAN, :].rearrange(
                            "a p t b -> p a (t b)"), dst[:])
                items.append(ydma)
            return items

        # -------- main loop: 3-stage pipeline with gap-filling interleave ----
        # iteration i: [deferred C(i-1)] | sweeps(i-1) with fillers:
        #   stage1 -> output chunks(i-2), stage2 -> U(i),
        #   stage3 -> C(i) for windows whose producers are already fixed
        # | tail: fixups(i-1).
        fixS_of = {}
        psums_of = {}
        started_of = {}
        fix_done = set()
        c_done = set()
        wins = _windows()
        emitted = set()
        order = []
        by_mw = {(m, w): (m, w, k0, L) for (m, w, k0, L) in wins}
        for (m, w, k0, L) in wins:
            if (m, w) in emitted:
                continue
            grp = [(m, w, k0, L)]
            emitted.add((m, w))
            nxt = (m, w + 1)
            if len(grp) < GRP and nxt in by_mw and nxt not in emitted:
                grp.append(by_mw[nxt])
                emitted.add(nxt)
            order.append(grp)
        NG = len(order)

        def producers_fixed(m, k0, L):
            for j in range(m + 1, M):
                r = 1 << (j - m)
                Lj = min(LE, T >> j)
                e_lo = max(0, k0 // r - 1)
                e_hi = (k0 + L) // r - 1
                for wj in range(e_lo // Lj, e_hi // Lj + 1):
                    if (j, wj) not in fix_done:
                        return False
            return True

        def do_Cfix(m, w, k0, L):
            fixS = emit_Cfix(m, w, k0, L, psums_of[(m, w)],
                             started_of.pop((m, w)))
            if fixS is not None:
                fixS_of[(m, w)] = fixS
            c_done.add((m, w))

        def do_fixups(g):
            for (m, w, k0, L) in order[g]:
                if w > 0:
                    emit_fixup(m, w, k0, L, psums_of[(m, w)])
                fix_done.add((m, w))

        def emit_all(items):
            for f in items:
                f()

        for i in range(NG + 2):
            out_items = []
            if i >= 2:
                its = []
                for (m, w, k0, L) in order[i - 2]:
                    its.append(emit_output_items(m, w, k0, L,
                                                 psums_of.pop((m, w))))
                while any(its):
                    for lst in its:
                        if lst:
                            out_items.append(lst.pop(0))

            if 1 <= i <= NG:
                grp = order[i - 1]
                for (m, w, k0, L) in grp:
                    if (m, w) not in c_done:
                        do_Cfix(m, w, k0, L)
                gens = [
                    _sweep_gen(nc, m, w, k0, L, psums_of[(m, w)], bias_sb,
                               whhT, sw_pool, v0f_pool, vper, v0fin)
                    for (m, w, k0, L) in grp
                ]
                for it in range(1, K + 1):
                    for g in gens:
                        next(g, None)
                    if it == 1:
                        half = (len(out_items) + 1) // 2
                        for f in out_items[:half]:
                            f()
                        out_items = out_items[half:]
                    elif it == 2:
                        emit_all(out_items)
                        out_items = []
                    elif it == 3 and i < NG:
                        for (m, w, k0, L) in order[i]:
                            psums_of[(m, w)], started_of[(m, w)] = \
                                emit_U(m, w, k0, L)
                    elif it == 4 and i < NG:
                        for (m, w, k0, L) in order[i]:
                            if producers_fixed(m, k0, L):
                                do_Cfix(m, w, k0, L)
                do_fixups(i - 1)
            else:
                emit_all(out_items)
                if i < NG:
                    for (m, w, k0, L) in order[i]:
                        psums_of[(m, w)], started_of[(m, w)] = \
                            emit_U(m, w, k0, L)

def _sweep_gen(nc, m, w, k0, L, Pp, bias_sb, whhT, sw_pool, v0f_pool,
               vper, v0fin):
    """Generator emitting one sweep stage per next() for lockstep pairing."""
    bias = bias_sb[:, m:m + 1]
    if m == 0:
        vfin = v0f_pool.tile([P, LE * BC], F16, tag="v0f",
                             name=f"v0f{w}")[:, :L * BC]
        v0fin[w] = vfin
        fin_ap = vfin
    else:
        fin_ap = vper[m][:, (k0 + 1) * BC:(k0 + 1 + L) * BC]
    sA = sw_pool.tile([P, LE * BC], F16, tag="swA", name=f"sA{m}_{w}")[:, :L * BC]
    sB = sw_pool.tile([P, LE * BC], F16, tag="swB", name=f"sB{m}_{w}")[:, :L * BC]
    sD = sw_pool.tile([P, LE * BC], F16, tag="swD", name=f"sD{m}_{w}")[:, :L * BC]
    lhsT = whhT[:, _WIDX[(m, m)], :]
    mm_groups = [(1, min(64, L))] + ([(64, L)] if L > 64 else [])

    def sweep_mm(rhs_buf, last):
        for (ka, kb) in mm_groups:
            nc.tensor.matmul(Pp[:, ka * BC:kb * BC], lhsT,
                             rhs_buf[:, (ka - 1) * BC:(kb - 1) * BC],
                             start=False, stop=last, skip_group_check=True)

    prev, cur = None, None  # v^{i-2}, v^{i-1} buffers
    for it in range(1, K + 1):
        last = it == K
        if it == 2:
            sweep_mm(cur, last)
        elif it > 2:
            nc.vector.tensor_tensor(sD[:, :(L - 1) * BC],
                                    cur[:, :(L - 1) * BC],
                                    prev[:, :(L - 1) * BC], SUB)
            sweep_mm(sD, last)
        out = fin_ap if last else (sA if it % 2 else sB)
        nc.scalar.activation(out, Pp[:, :L * BC], TANH, bias=bias, scale=1.0)
        prev, cur = cur, out
        yield


_NC_CACHE = None


def _prep_weights(inputs):
    wih = np.asarray(inputs["weight_ih"], dtype=np.float32)
    whh = np.asarray(inputs["weight_hh"], dtype=np.float32)
    fcw = np.asarray(inputs["fc_w"], dtype=np.float32)
    wihT = np.ascontiguousarray(
        wih.reshape(M, MS, 2, P).transpose(3, 2, 0, 1).astype(np.float16))
    whhT = np.empty((P, NBLK, P), dtype=np.float16)
    for (j, m), idx in _WIDX.items():
        whhT[:, idx, :] = whh[m * MS:(m + 1) * MS, j * MS:(j + 1) * MS].T
    fcwT = np.ascontiguousarray(
        fcw.reshape(2, P, M, MS).transpose(3, 2, 0, 1).astype(np.float16))
    biasb = np.ascontiguousarray(
        (np.asarray(inputs["bias_ih"], dtype=np.float32)
         + np.asarray(inputs["bias_hh"], dtype=np.float32))
        .reshape(M, P).T)
    fcbb = np.ascontiguousarray(
        np.asarray(inputs["fc_b"], dtype=np.float32).reshape(2, P).T)
    return dict(wihT=wihT, whhT=whhT, fcwT=fcwT, biasb=biasb, fcbb=fcbb)


def _prep_x(x_core):
    """[BC, T, I] fp32 -> x^T [2, P, T, BC] fp16 (+ mid-rate tensor)."""
    xt = np.ascontiguousarray(
        x_core.transpose(2, 1, 0).astype(np.float16).reshape(2, P, T, BC))
    xmid = np.ascontiguousarray(xt[:, :, ::4, :])
    return xt, xmid


def kernel(**inputs):
    global _NC_CACHE
    x = np.asarray(inputs["x"], dtype=np.float32)
    assert int(np.asarray(inputs["n_modules"])) == M
    weights = _prep_weights(inputs)
    if _NC_CACHE is None:
        _NC_CACHE = build_nc()
    nc = _NC_CACHE
    in_maps = []
    for c in range(CORES):
        xt, xmid = _prep_x(x[c * BC:(c + 1) * BC])
        in_maps.append(dict(x=xt, xmid=xmid, **weights))
    res = run_bass_kernel_spmd(nc, in_maps, list(range(CORES)))
    out = np.empty((B, T, I), dtype=np.float32)
    for c in range(CORES):
        yt = res.results[c]["y"]  # [2, P, T, BC] fp16
        out[c * BC:(c + 1) * BC] = \
            yt.reshape(I, T, BC).transpose(2, 1, 0).astype(np.float32)
    return out


if __name__ == "__main__":
    build_nc()
    print("built OK")


# revision 26
# speedup vs baseline: 1.2029x; 1.1376x over previous
"""Trainium2 Bass kernel for CwRNN (nn_CwRNN_84971632984686).

Data-parallel over batch (64/8 = 8 rows per core). Per core:
- Host pre-transposes x to x^T [2ic, 128i, T, 8b] fp16 (plus a T/4-rate
  mid tensor) and pre-transposes all weights, so the device does zero
  transposes/casts; y is produced transposed fp16 and the host restores
  [b, T, I] fp32.
- Module-decoupled clockwork solve (block-triangular W_hh): module m
  depends only on modules >= m. Self-recurrence solved per 128-entry
  window by parallel-in-time Jacobi (K tanh passes, delta-accumulated
  in a persistent PSUM window).
- Windows use a ZERO boundary (fully independent, schedulable in
  lockstep pairs to keep ACT/PE saturated); a tiny FIX-entry fixup pass
  with the exact boundary restores the first entries afterwards.
  Boundary influence decays ~0.45^k so entries >= FIX are unaffected.
- Output y^T built per 64-step chunk in PSUM: G_m = fc_w,m^T @ v_m plus
  an upsample-add of the parent coarse level (c-hierarchy kept in SBUF
  fp16; the up2-add is fused into the PSUM->SBUF copy on DVE/Pool).
- Columns are entry-major (col = k*8 + b) so all broadcasts/upsamples
  are uniform access patterns.
"""
import os
import sys
import numpy as np

for _p in ("/root/.axon_site/_ro/trn_rl_repo", "/opt/trn_rl_repo"):
    if os.path.isdir(_p) and _p not in sys.path:
        sys.path.insert(0, _p)

import concourse.bass as bass  # noqa: E402
import concourse.mybir as mybir  # noqa: E402
from concourse import bacc  # noqa: E402
from concourse.tile import TileContext  # noqa: E402
from concourse.masks import make_identity  # noqa: E402
from concourse.bass_utils import run_bass_kernel_spmd  # noqa: E402

F32 = mybir.dt.float32
F16 = mybir.dt.float16
TANH = mybir.ActivationFunctionType.Tanh
COPYF = mybir.ActivationFunctionType.Copy
ADD = mybir.AluOpType.add
SUB = mybir.AluOpType.subtract

CORES = 8
B, T, I, H, M = 64, 2048, 256, 1024, 8
MS = H // M          # 128, module size
BC = B // CORES      # 8 batch rows per core
P = 128
LE = 128             # max entries per solve window
K = 3                # Jacobi sweeps (tanh passes)
FIX = 8              # fixup entries per window boundary
KF = 3               # fixup sweeps
SPAN = 128           # steps per x/y span tile
GRP = 2              # windows solved in lockstep
TM4 = T // 4

_WIDX = {}
for _m in range(M):
    for _j in range(_m, M):
        _WIDX[(_j, _m)] = len(_WIDX)
NBLK = len(_WIDX)


def _run_segments(ka, kb, r):
    """Segments covering ks [max(ka,1), kb): list of (k_start, slot_start,
    n, rep): out cols [k_start, k_start+n*rep) read slots [slot_start,
    slot_start+n) broadcast rep-wide. ceil(k/r) indexing."""
    segs = []
    k = max(ka, 1)
    while k < kb:
        s = (k + r - 1) // r
        run_end = min(kb, s * r + 1)
        ln = run_end - k
        if ln == r:
            nfull = (kb - k) // r
            segs.append((k, s, nfull, r))
            k += nfull * r
        else:
            segs.append((k, s, 1, ln))
            k = run_end
    return segs


def _windows():
    ws = []
    for m in range(M):
        Tm = T >> m
        L = min(LE, Tm)
        for w in range(Tm // L):
            ws.append((m, w, w * L, L))
    ws.sort(key=lambda s: (s[2] * (1 << s[0]), -s[0]))
    return ws


def build_nc():
    nc = bacc.Bacc("TRN2", target_bir_lowering=False, debug=False)
    dr = {}
    dr["x"] = nc.dram_tensor("x", [2, P, T, BC], F16, kind="ExternalInput")
    dr["xmid"] = nc.dram_tensor("xmid", [2, P, TM4, BC], F16,
                                kind="ExternalInput")
    dr["wihT"] = nc.dram_tensor("wihT", [P, 2, M, P], F16,
                                kind="ExternalInput")
    dr["whhT"] = nc.dram_tensor("whhT", [P, NBLK, P], F16,
                                kind="ExternalInput")
    dr["fcwT"] = nc.dram_tensor("fcwT", [P, M, 2, P], F16,
                                kind="ExternalInput")
    dr["biasb"] = nc.dram_tensor("biasb", [P, M], F32, kind="ExternalInput")
    dr["fcbb"] = nc.dram_tensor("fcbb", [P, 2], F32, kind="ExternalInput")
    dr["y"] = nc.dram_tensor("y", [2, P, T, BC], F16, kind="ExternalOutput")
    with TileContext(nc) as tc:
        _emit(tc, nc, dr)
    nc.compile()
    return nc


def _emit(tc, nc, dr):
    import contextlib
    ctx = contextlib.ExitStack()
    with ctx:
        cst = ctx.enter_context(tc.tile_pool(name="cst", bufs=1))
        xsp_pool = ctx.enter_context(tc.tile_pool(name="xsp", bufs=8))
        vper_pool = ctx.enter_context(tc.tile_pool(name="vper", bufs=1))
        v0f_pool = ctx.enter_context(tc.tile_pool(name="v0f", bufs=6))
        sw_pool = ctx.enter_context(tc.tile_pool(name="sw", bufs=3))
        fx_pool = ctx.enter_context(tc.tile_pool(name="fx", bufs=3))
        cch_pool = ctx.enter_context(tc.tile_pool(name="cch", bufs=4))
        yst_pool = ctx.enter_context(tc.tile_pool(name="yst", bufs=2))
        pw = ctx.enter_context(tc.tile_pool(name="pw", bufs=4, space="PSUM"))

        # ---------------- constants (no transposes: host pre-transposed) ---
        ident = cst.tile([P, P], F32)
        make_identity(nc, ident)
        ident16 = cst.tile([P, P], F16)
        nc.vector.tensor_copy(ident16[:], ident[:])

        wihT = cst.tile([P, 2, M, P], F16)
        nc.sync.dma_start(wihT[:], dr["wihT"][:, :, :, :])
        whhT = cst.tile([P, NBLK, P], F16)
        nc.sync.dma_start(whhT[:], dr["whhT"][:, :, :])
        fcwT = cst.tile([P, M, 2, P], F16)
        nc.sync.dma_start(fcwT[:], dr["fcwT"][:, :, :, :])
        bias_sb = cst.tile([P, M], F32)
        nc.sync.dma_start(bias_sb[:], dr["biasb"][:, :])
        fcb_sb = cst.tile([P, 2], F32)
        nc.sync.dma_start(fcb_sb[:], dr["fcbb"][:, :])
        xmid = cst.tile([P, 2, TM4 * BC], F16)
        xmv = dr["xmid"][:, :, :, :].rearrange("a p t b -> p a (t b)")
        nc.sync.dma_start(xmid[:, 0, :], xmv[:, 0, :])
        nc.scalar.dma_start(xmid[:, 1, :], xmv[:, 1, :])

        # persistent per-module finals, slot s = entry s-1 (slot 0 = zero)
        vper = {}
        for m in range(1, M):
            Tm = T >> m
            vper[m] = vper_pool.tile([P, (Tm + 1) * BC], F16, tag=f"vp{m}",
                                     name=f"vp{m}")
            nc.vector.memset(vper[m][:, 0:BC], 0.0)

        # coarse output levels for single-window modules (computed once)
        csing = {}
        for m in range(4, M):
            Tm = T >> m
            csing[m] = cst.tile([P, 2, Tm * BC], F32, name=f"c{m}")

        xtiles = {}

        def load_span(s):
            if s in xtiles:
                return
            t0 = xsp_pool.tile([P, 2, SPAN * BC], F16, tag="xsp", name="xt")
            nc.sync.dma_start(
                t0[:],
                dr["x"][:, :, s * SPAN:(s + 1) * SPAN, :].rearrange(
                    "a p t b -> p a (t b)"))
            xtiles[s] = t0

        for _s in range(6):
            load_span(_s)

        v0fin = {}      # w -> m=0 final tile [P, L*BC] (entry k at col k*8)
        cchunk = {}     # (m, w) -> c_m chunk tile [P, 2, L*BC]
        copy_ctr = [0]

        def copy_engine():
            # Pool/GPSIMD cannot access PSUM on HW; DVE handles fused copies
            copy_ctr[0] += 1
            return nc.vector

        def u_rhs(m, w, ic, ka, kb):
            """x^T RHS for window entries [ka, kb) (module-local)."""
            k0 = w * (min(LE, T >> m))
            if m == 0:
                xs = xtiles[w][:, ic, :].rearrange("p (t b) -> p t b", b=BC)
                return xs[:, ka:kb, :]
            if m == 1:
                tile = xtiles[2 * w + (ka // 64)]
                xs = tile[:, ic, :].rearrange("p (t s b) -> p t s b",
                                              s=2, b=BC)
                return xs[:, 0:kb - ka, 0, :]
            stride = 1 << (m - 2)
            xm = xmid[:, ic, :].rearrange("p (k s b) -> p k s b",
                                          s=stride, b=BC)
            return xm[:, k0 + ka:k0 + kb, 0, :]

        def emit_C(m, w, k0, L, Pf, started):
            """P[:, k*8+b] += sum_{j>m} W_mj v_j[slot k0/r + ceil(k/r)]."""
            groups = [(0, min(64, L))] + ([(64, L)] if L > 64 else [])
            for j in range(m + 1, M):
                r = 1 << (j - m)
                sb = k0 // r
                lhsT = whhT[:, _WIDX[(j, m)], :]
                Vj = vper[j][:].rearrange("p (s b) -> p s b", b=BC)
                for (ka, kb) in groups:
                    gi = ka // 64

                    def mm(c0, c1, rhs):
                        st = gi not in started
                        if st:
                            started.add(gi)
                        nc.tensor.matmul(Pf[:, c0 * BC:c1 * BC], lhsT, rhs,
                                         start=st, stop=False,
                                         skip_group_check=True)

                    if ka == 0:
                        mm(0, 1, Vj[:, sb:sb + 1, :])
                    for (k, s, n, rep) in _run_segments(ka, kb, r):
                        rhs = Vj[:, sb + s:sb + s + n, :][
                            :, :, None, :].broadcast_to((P, n, rep, BC))
                        mm(k, k + n * rep, rhs)

        ch_of = {}

        def emit_Chalf_pre(m, w, k0, Pp):
            """Coarse C at half rate into Pp[:, 512:1024); copy to SBUF."""
            first = [True]
            for j in range(m + 1, M):
                rh = 1 << (j - m - 1)
                sb = k0 // (rh * 2)
                lhsT = whhT[:, _WIDX[(j, m)], :]
                Vj = vper[j][:].rearrange("p (s b) -> p s b", b=BC)
                for (i, s, n, rep) in _run_segments(1, 65, rh):
                    rhs = Vj[:, sb + s:sb + s + n, :][
                        :, :, None, :].broadcast_to((P, n, rep, BC))
                    nc.tensor.matmul(
                        Pp[:, 512 + (i - 1) * BC:512 + (i - 1 + n * rep) * BC],
                        lhsT, rhs, start=first[0], stop=False,
                        skip_group_check=True)
                    first[0] = False
            ch = fx_pool.tile([P, 512], F16, tag="ch", name="ch")
            nc.scalar.activation(ch[:], Pp[:, 512:1024], COPYF, bias=0.0,
                                 scale=1.0)
            ch_of[(m, w)] = ch

        def emit_Chalf_post(m, w, k0, Pp):
            """k=0 boundary singles + up2 of coarse C into the window."""
            ch = ch_of.pop((m, w))
            Chv = ch[:].rearrange("p (i b) -> p i b", b=BC)
            for j in range(m + 1, M):
                r = 1 << (j - m)
                sb = k0 // r
                lhsT = whhT[:, _WIDX[(j, m)], :]
                Vj = vper[j][:].rearrange("p (s b) -> p s b", b=BC)
                nc.tensor.matmul(Pp[:, 0:BC], lhsT, Vj[:, sb:sb + 1, :],
                                 start=False, stop=False,
                                 skip_group_check=True)
            for (ka, kb) in ((0, 64), (64, 128)):
                for (k, s, n, rep) in _run_segments(ka, kb, 2):
                    rhs = Chv[:, s - 1:s - 1 + n, :][
                        :, :, None, :].broadcast_to((P, n, rep, BC))
                    nc.tensor.matmul(Pp[:, k * BC:(k + n * rep) * BC],
                                     ident16[:], rhs, start=False,
                                     stop=False, skip_group_check=True)

        def emit_U(m, w, k0, L):
            Pp = pw.tile([P, LE * BC], F32, tag="pw",
                         name=f"P{m}_{w}")[:, :L * BC]
            started = set()
            groups = [(0, min(64, L))] + ([(64, L)] if L > 64 else [])
            if m == 0:
                load_span(w)
            elif m == 1:
                load_span(2 * w)
                load_span(2 * w + 1)
            half = m <= 3 and producers_fixed(m, k0, L)
            if half:
                emit_Chalf_pre(m, w, k0, Pp)
            for ic in range(2):
                for (ka, kb) in groups:
                    gi = ka // 64
                    st = gi not in started
                    if st:
                        started.add(gi)
                    nc.tensor.matmul(Pp[:, ka * BC:kb * BC],
                                     wihT[:, ic, m, :], u_rhs(m, w, ic, ka, kb),
                                     start=st, stop=False,
                                     skip_group_check=True)
            if half:
                emit_Chalf_post(m, w, k0, Pp)
                started = None
            return Pp, started

        def emit_Cfix(m, w, k0, L, Pp, started):
            if started is not None:
                emit_C(m, w, k0, L, Pp, started)
            fixS = None
            if w > 0:
                fixS = fx_pool.tile([P, FIX * BC], F16, tag="fS", name="fS")
                nc.vector.tensor_copy(fixS[:], Pp[:, :FIX * BC])
            return fixS

        def emit_fixup(m, w, k0, L, Pp):
            """Redo entries [0..FIX) with exact boundary from window w-1.
            Reuses the window's own (dead) PSUM columns for the fixup."""
            if m == 0:
                bnd = v0fin[w - 1][:, (L - 1) * BC:L * BC]
                main = v0fin[w]
                main_head = main[:, :(FIX - 1) * BC]
                fin = main[:, :FIX * BC]
            else:
                Vm = vper[m]
                bnd = Vm[:, k0 * BC:(k0 + 1) * BC]
                main_head = Vm[:, (k0 + 1) * BC:(k0 + FIX) * BC]
                fin = Vm[:, (k0 + 1) * BC:(k0 + 1 + FIX) * BC]
            fixS = fixS_of.pop((m, w))
            lhsT = whhT[:, _WIDX[(m, m)], :]
            bias = bias_sb[:, m:m + 1]
            PF = Pp[:, :FIX * BC]
            nc.tensor.matmul(PF, ident16[:], fixS[:], start=True,
                             stop=False, skip_group_check=True)
            nc.tensor.matmul(PF[:, 0:BC], lhsT, bnd, start=False, stop=False,
                             skip_group_check=True)
            nc.tensor.matmul(PF[:, BC:FIX * BC], lhsT, main_head,
                             start=False, stop=False, skip_group_check=True)
            fA = fx_pool.tile([P, FIX * BC], F16, tag="fA", name="fA")
            fB = fx_pool.tile([P, FIX * BC], F16, tag="fB", name="fB")
            fD = fx_pool.tile([P, (FIX - 1) * BC], F16, tag="fD", name="fD")
            nc.scalar.activation(fA[:], PF, TANH, bias=bias, scale=1.0)
            prev_src, cur_buf = main_head, fA
            for it in range(2, KF + 1):
                last = it == KF
                nc.vector.tensor_tensor(fD[:], cur_buf[:, :(FIX - 1) * BC],
                                        prev_src, SUB)
                nc.tensor.matmul(PF[:, BC:FIX * BC], lhsT, fD[:],
                                 start=False, stop=last,
                                 skip_group_check=True)
                out = fin if last else (fB if cur_buf is fA else fA)
                nc.scalar.activation(out, PF, TANH, bias=bias, scale=1.0)
                prev_src = cur_buf[:, :(FIX - 1) * BC]
                cur_buf = out

        def emit_output_items(m, w, k0, L, Pp):
            """Returns closures: G_m chunks + fused up2-add into c_m / y
            (reuses the window's PSUM)."""
            ngr = max(1, L // 64)
            if m == 0:
                dst = yst_pool.tile([P, 2, SPAN * BC], F16, tag="yst",
                                    name=f"yst{w}")
                par = cchunk[(1, w // 2)]
                pk0 = (w // 2) * LE
            elif m >= 4:
                dst = csing[m]
                par = csing[m + 1] if m < M - 1 else None
                pk0 = 0
            else:
                dst = cch_pool.tile([P, 2, L * BC], F16, tag=f"c{m}",
                                    name=f"c{m}_{w}")
                cchunk[(m, w)] = dst
                par = csing[4] if m == 3 else cchunk[(m + 1, w // 2)]
                pk0 = 0 if m == 3 else (w // 2) * LE
            items = []
            for ic in range(2):
                for g in range(ngr):
                    def chunk(ic=ic, g=g):
                        ka, kb = 64 * g, min(64 * (g + 1), L)
                        ncols = (kb - ka) * BC
                        off = (512 * ((ic * ngr + g) % 2)
                               if L * BC >= 1024 else 0)
                        g_ps = Pp[:, off:off + ncols]
                        if m == 0:
                            rhs = v0fin[w][:, ka * BC:kb * BC]
                        else:
                            rhs = vper[m][:, (k0 + 1 + ka) * BC:
                                          (k0 + 1 + kb) * BC]
                        nc.tensor.matmul(g_ps, fcwT[:, m, ic, :], rhs,
                                         start=True, stop=True,
                                         skip_group_check=True)
                        out = dst[:, ic, ka * BC:kb * BC]
                        if m == M - 1:
                            nc.vector.tensor_scalar_add(out, g_ps,
                                                        fcb_sb[:, ic:ic + 1])
                        else:
                            e0 = (k0 + ka) // 2 - pk0
                            ne = (kb - ka) // 2
                            pv = par[:, ic, :].rearrange("p (e b) -> p e b",
                                                         b=BC)
                            rhs2 = pv[:, e0:e0 + ne, :][:, :, None, :] \
                                .broadcast_to((P, ne, 2, BC))
                            copy_engine().tensor_tensor(out, g_ps, rhs2, ADD)
                    items.append(chunk)
            if m == 0:
                def ydma():
                    nc.gpsimd.dma_start(
                        dr["y"][:, :, w * SPAN:(w + 1) * SPAN, :].rearrange(
                            "a p t b -> p a (t b)"), dst[:])
                items.append(ydma)
            return items

        # -------- main loop: 3-stage pipeline with gap-filling interleave ----
        # iteration i: [deferred C(i-1)] | sweeps(i-1) with fillers:
        #   stage1 -> output chunks(i-2), stage2 -> U(i),
        #   stage3 -> C(i) for windows whose producers are already fixed
        # | tail: fixups(i-1).
        fixS_of = {}
        psums_of = {}
        started_of = {}
        fix_done = set()
        c_done = set()
        wins = _windows()
        emitted = set()
        order = []
        by_mw = {(m, w): (m, w, k0, L) for (m, w, k0, L) in wins}
        for (m, w, k0, L) in wins:
            if (m, w) in emitted:
                continue
            grp = [(m, w, k0, L)]
            emitted.add((m, w))
            nxt = (m, w + 1)
            if len(grp) < GRP and nxt in by_mw and nxt not in emitted:
                grp.append(by_mw[nxt])
                emitted.add(nxt)
            order.append(grp)
        NG = len(order)

        def producers_fixed(m, k0, L):
            for j in range(m + 1, M):
                r = 1 << (j - m)
                Lj = min(LE, T >> j)
                e_lo = max(0, k0 // r - 1)
                e_hi = (k0 + L) // r - 1
                for wj in range(e_lo // Lj, e_hi // Lj + 1):
                    if (j, wj) not in fix_done:
                        return False
            return True

        def do_Cfix(m, w, k0, L):
            fixS = emit_Cfix(m, w, k0, L, psums_of[(m, w)],
                             started_of.pop((m, w)))
            if fixS is not None:
                fixS_of[(m, w)] = fixS
            c_done.add((m, w))

        def do_fixups(g):
            for (m, w, k0, L) in order[g]:
                if w > 0:
                    emit_fixup(m, w, k0, L, psums_of[(m, w)])
                fix_done.add((m, w))

        def emit_all(items):
            for f in items:
                f()

        for i in range(NG + 2):
            out_items = []
            if i >= 2:
                its = []
                for (m, w, k0, L) in order[i - 2]:
                    its.append(emit_output_items(m, w, k0, L,
                                                 psums_of.pop((m, w))))
                while any(its):
                    for lst in its:
                        if lst:
                            out_items.append(lst.pop(0))

            if 1 <= i <= NG:
                grp = order[i - 1]
                for (m, w, k0, L) in grp:
                    if (m, w) not in c_done:
                        do_Cfix(m, w, k0, L)
                gens = [
                    _sweep_gen(nc, m, w, k0, L, psums_of[(m, w)], bias_sb,
                               whhT, sw_pool, v0f_pool, vper, v0fin)
                    for (m, w, k0, L) in grp
                ]
                for it in range(1, K + 1):
                    for g in gens:
                        next(g, None)
                    if it == 1:
                        half = (len(out_items) + 1) // 2
                        for f in out_items[:half]:
                            f()
                        out_items = out_items[half:]
                    elif it == 2:
                        emit_all(out_items)
                        out_items = []
                    elif it == 3 and i < NG:
                        for (m, w, k0, L) in order[i]:
                            psums_of[(m, w)], started_of[(m, w)] = \
                                emit_U(m, w, k0, L)
                    elif it == 4 and i < NG:
                        for (m, w, k0, L) in order[i]:
                            if producers_fixed(m, k0, L):
                                do_Cfix(m, w, k0, L)
                do_fixups(i - 1)
            else:
                emit_all(out_items)
                if i < NG:
                    for (m, w, k0, L) in order[i]:
                        psums_of[(m, w)], started_of[(m, w)] = \
                            emit_U(m, w, k0, L)

def _sweep_gen(nc, m, w, k0, L, Pp, bias_sb, whhT, sw_pool, v0f_pool,
               vper, v0fin):
    """Generator emitting one sweep stage per next() for lockstep pairing."""
    bias = bias_sb[:, m:m + 1]
    if m == 0:
        vfin = v0f_pool.tile([P, LE * BC], F16, tag="v0f",
                             name=f"v0f{w}")[:, :L * BC]
        v0fin[w] = vfin
        fin_ap = vfin
    else:
        fin_ap = vper[m][:, (k0 + 1) * BC:(k0 + 1 + L) * BC]
    sA = sw_pool.tile([P, LE * BC], F16, tag="swA", name=f"sA{m}_{w}")[:, :L * BC]
    sB = sw_pool.tile([P, LE * BC], F16, tag="swB", name=f"sB{m}_{w}")[:, :L * BC]
    sD = sw_pool.tile([P, LE * BC], F16, tag="swD", name=f"sD{m}_{w}")[:, :L * BC]
    lhsT = whhT[:, _WIDX[(m, m)], :]
    mm_groups = [(1, min(64, L))] + ([(64, L)] if L > 64 else [])

    def sweep_mm(rhs_buf, last):
        for (ka, kb) in mm_groups:
            nc.tensor.matmul(Pp[:, ka * BC:kb * BC], lhsT,
                             rhs_buf[:, (ka - 1) * BC:(kb - 1) * BC],
                             start=False, stop=last, skip_group_check=True)

    prev, cur = None, None  # v^{i-2}, v^{i-1} buffers
    for it in range(1, K + 1):
        last = it == K
        if it == 2:
            sweep_mm(cur, last)
        elif it > 2:
            nc.vector.tensor_tensor(sD[:, :(L - 1) * BC],
                                    cur[:, :(L - 1) * BC],
                                    prev[:, :(L - 1) * BC], SUB)
            sweep_mm(sD, last)
        out = fin_ap if last else (sA if it % 2 else sB)
        nc.scalar.activation(out, Pp[:, :L * BC], TANH, bias=bias, scale=1.0)
        prev, cur = cur, out
        yield


_NC_CACHE = None


def _prep_weights(inputs):
    wih = np.asarray(inputs["weight_ih"], dtype=np.float32)
    whh = np.asarray(inputs["weight_hh"], dtype=np.float32)
    fcw = np.asarray(inputs["fc_w"], dtype=np.float32)
    wihT = np.ascontiguousarray(
        wih.reshape(M, MS, 2, P).transpose(3, 2, 0, 1).astype(np.float16))
    whhT = np.empty((P, NBLK, P), dtype=np.float16)
    for (j, m), idx in _WIDX.items():
        whhT[:, idx, :] = whh[m * MS:(m + 1) * MS, j * MS:(j + 1) * MS].T
    fcwT = np.ascontiguousarray(
        fcw.reshape(2, P, M, MS).transpose(3, 2, 0, 1).astype(np.float16))
    biasb = np.ascontiguousarray(
        (np.asarray(inputs["bias_ih"], dtype=np.float32)
         + np.asarray(inputs["bias_hh"], dtype=np.float32))
        .reshape(M, P).T)
    fcbb = np.ascontiguousarray(
        np.asarray(inputs["fc_b"], dtype=np.float32).reshape(2, P).T)
    return dict(wihT=wihT, whhT=whhT, fcwT=fcwT, biasb=biasb, fcbb=fcbb)


def _prep_x(x_core):
    """[BC, T, I] fp32 -> x^T [2, P, T, BC] fp16 (+ mid-rate tensor)."""
    xt = np.ascontiguousarray(
        x_core.transpose(2, 1, 0).astype(np.float16).reshape(2, P, T, BC))
    xmid = np.ascontiguousarray(xt[:, :, ::4, :])
    return xt, xmid


def kernel(**inputs):
    global _NC_CACHE
    x = np.asarray(inputs["x"], dtype=np.float32)
    assert int(np.asarray(inputs["n_modules"])) == M
    weights = _prep_weights(inputs)
    if _NC_CACHE is None:
        _NC_CACHE = build_nc()
    nc = _NC_CACHE
    in_maps = []
    for c in range(CORES):
        xt, xmid = _prep_x(x[c * BC:(c + 1) * BC])
        in_maps.append(dict(x=xt, xmid=xmid, **weights))
    res = run_bass_kernel_spmd(nc, in_maps, list(range(CORES)))
    out = np.empty((B, T, I), dtype=np.float32)
    for c in range(CORES):
        yt = res.results[c]["y"]  # [2, P, T, BC] fp16
        out[c * BC:(c + 1) * BC] = \
            yt.reshape(I, T, BC).transpose(2, 1, 0).astype(np.float32)
    return out


if __name__ == "__main__":
    build_nc()
    print("built OK")
